# revision 20
# baseline (speedup 1.0000x reference)
"""GroupedQueryAttention TRN2 Bass kernel (v2).

Strategy (8 NeuronCores, tensor-parallel over heads):
  - Each core owns 4 q-heads (one kv head, GQA group of 4), all tokens.
  - Phase 1: QKV projection (bf16 matmuls, N=512 chunks) + fused RoPE.
    Q stored [64, 4 heads, NI] so scores batch 2 heads per matmul.
  - Phase 3: causal flash-style attention per (head-pair, batch, q-stripe):
      S = K_blk^T.T @ Q(2 heads)  -> exp on ACT (causally trimmed)
      ctx^T += V_aug.T @ exp  (V augmented with ones column so the softmax
      denominator falls out of the same matmul); normalize via reciprocal +
      partition broadcast fused into the bf16 staging store.
  - Phase 4: two 1 MB AllToAlls (one per head-pair) re-shard from
    head-sharded to token-sharded; the first overlaps pair-1 attention.
  - Phase 5: out = ctx_tok^T stationary x full-Wo moving (N=2048 matmuls),
    each core emits out[token-slice 512, 2048].
  - Host concatenates the 8 token slices.
"""

import os
import sys

import numpy as np


def _ensure_concourse():
    try:
        import concourse.bass  # noqa: F401
    except ImportError:
        for p in ("/opt/trn_rl_repo", "/root/.axon_site/_ro/trn_rl_repo"):
            if os.path.isdir(p) and p not in sys.path:
                sys.path.insert(0, p)
        import concourse.bass  # noqa: F401


FULL_CFG = dict(B=2, S=2048, E=2048, NH=32, NKV=8, HD=64, ncores=8, IC=512)

LAST_RESULTS = None
_CACHED_NC = None


def build_gqa(cfg):
    """Build the Bass module for one core's SPMD program. Returns nc."""
    _ensure_concourse()
    from contextlib import ExitStack

    import concourse.mybir as mybir
    import concourse.tile as tile
    from concourse import bacc
    from concourse.masks import make_identity

    dt = mybir.dt
    f32 = dt.float32
    bf16 = dt.bfloat16
    Exp = mybir.ActivationFunctionType.Exp

    B, S, E = cfg["B"], cfg["S"], cfg["E"]
    NH, NKV, HD = cfg["NH"], cfg["NKV"], cfg["HD"]
    NCORES = cfg["ncores"]
    HPC = NH // NCORES          # 4 q heads per core
    assert HPC == 4 and HD == 64
    QH = HPC * HD               # 256 ctx rows per core
    KVD = 2 * HD                # 128 packed K|V projection width
    NI = B * S                  # 4096 tokens
    ET = E // 128               # 16 contraction tiles
    IC = cfg["IC"]              # phase-1 token chunk (1024)
    QB = 512                    # attention q stripe
    KB = 128                    # attention k block
    NQT = S // QB               # 4 stripes per batch
    SKT = S // KB               # 16 k tiles per batch
    NKTILES = NI // KB          # 32 k tiles
    TOK = NI // NCORES          # 512-token output slice per core
    scale = 1.0 / float(np.sqrt(HD))

    nc = bacc.Bacc("TRN2", target_bir_lowering=False, debug=False,
                   num_devices=NCORES)

    xT = nc.dram_tensor("xT", [E, NI], bf16, kind="ExternalInput").ap()
    wqT = nc.dram_tensor("wqT", [E, QH], bf16, kind="ExternalInput").ap()
    wkvT = nc.dram_tensor("wkvT", [E, KVD], bf16, kind="ExternalInput").ap()
    woT = nc.dram_tensor("woT", [E, E], bf16, kind="ExternalInput").ap()
    cosT = nc.dram_tensor("cosT", [128, S], bf16, kind="ExternalInput").ap()
    sinT = nc.dram_tensor("sinT", [128, S], bf16, kind="ExternalInput").ap()
    outT = nc.dram_tensor("outT", [TOK, E], bf16, kind="ExternalOutput").ap()

    with tile.TileContext(nc) as tc, ExitStack() as persist:
        const = persist.enter_context(tc.tile_pool(name="const", bufs=1))
        qt_pool = persist.enter_context(tc.tile_pool(name="qt", bufs=1))
        kt_pool = persist.enter_context(tc.tile_pool(name="kt", bufs=1))
        vaug_pool = persist.enter_context(tc.tile_pool(name="vaug", bufs=1))
        dram = persist.enter_context(
            tc.tile_pool(name="dram", bufs=1, space="DRAM"))

        ident = const.tile([128, 128], bf16, name="ident", tag="ident")
        make_identity(nc, ident[:, :])
        # wq + x chunks go on the sync DGE ring; cos/sin/wo on the scalar
        # ring so the 8 MB wo load does not delay the first matmul.
        wq_sb = const.tile([128, ET, QH], bf16, name="wq_sb", tag="wq")

        def load_wq_quarter(ts):
            nc.sync.dma_start(
                wq_sb[:, ts:ts + 4, :],
                wqT[ts * 128:(ts + 4) * 128, :].rearrange(
                    "(t p) o -> p t o", p=128))

        load_wq_quarter(0)
        wkv_sb = const.tile([128, ET, KVD], bf16, name="wkv_sb", tag="wkv")
        nc.sync.dma_start(wkv_sb[:, :, :],
                          wkvT.rearrange("(t p) o -> p t o", p=128))
        cos_sb = const.tile([128, S], bf16, name="cos_sb", tag="cos")
        nc.scalar.dma_start(cos_sb[:, :], cosT)
        sin_sb = const.tile([128, S], bf16, name="sin_sb", tag="sin")
        nc.scalar.dma_start(sin_sb[:, :], sinT)
        # triangular causal mask for the diagonal 128-block, dup for 2 heads
        tri = const.tile([128, 2, 128], bf16, name="tri", tag="tri")
        nc.gpsimd.memset(tri[:, :, :], 1.0)
        nc.gpsimd.affine_select(
            out=tri[:, :, :], in_=tri[:, :, :],
            pattern=[[0, 2], [1, 128]], compare_op=mybir.AluOpType.is_ge,
            fill=0.0, base=0, channel_multiplier=-1)

        # persistent activations
        qt_sb = qt_pool.tile([64, HPC, NI], bf16, name="qt", tag="qt")
        kt_sb = kt_pool.tile([64, NI], bf16, name="kt", tag="kt")
        vaug = [vaug_pool.tile([128, 2 * HD], bf16, name=f"va{k}",
                               tag=f"va{k}")
                for k in range(NKTILES)]
        for k in range(NKTILES):
            nc.vector.memset(vaug[k][:, :], 0.0)
            nc.vector.memset(vaug[k][:, 0:1], 1.0)

        # collective buffers: per head-pair m, [slice, 128 rows, 512 tokens]
        cc_in = [dram.tile([NCORES, 128, TOK], bf16, name=f"cc_in{m}",
                           tag=f"ccin{m}") for m in range(2)]
        cc_out = [dram.tile([NCORES, 128, TOK], bf16, name=f"cc_out{m}",
                            tag=f"ccout{m}") for m in range(2)]

        # ---- phase 1: QKV projection + RoPE + V transpose
        with ExitStack() as ph1:
            xt_pool = ph1.enter_context(tc.tile_pool(name="xt", bufs=2))
            proj_ps = ph1.enter_context(
                tc.tile_pool(name="proj_ps", bufs=3, space="PSUM"))
            vt_ps_pool = ph1.enter_context(
                tc.tile_pool(name="vt_ps", bufs=2, space="PSUM"))
            rope_pool = ph1.enter_context(tc.tile_pool(name="rope", bufs=4))
            vs_pool = ph1.enter_context(tc.tile_pool(name="vs", bufs=3))

            def rope(src_ps, parts, s0, dsts):
                # dsts: list of (out_ap, row0) pairs covering src rows
                t1 = rope_pool.tile([128, IC], bf16, name="t1", tag="t1")
                sw = rope_pool.tile([128, IC], bf16, name="sw", tag="sw")
                for h0 in range(0, parts, 64):
                    nc.scalar.copy(sw[h0:h0 + 32, :],
                                   src_ps[h0 + 32:h0 + 64, :])
                    nc.scalar.copy(sw[h0 + 32:h0 + 64, :],
                                   src_ps[h0:h0 + 32, :])
                nc.vector.tensor_mul(t1[:parts, :], src_ps[:parts, :],
                                     cos_sb[:parts, s0:s0 + IC])
                nc.vector.tensor_mul(sw[:parts, :], sw[:parts, :],
                                     sin_sb[:parts, s0:s0 + IC])
                for out_ap, r0 in dsts:
                    nc.vector.tensor_add(out_ap, t1[r0:r0 + 64, :],
                                         sw[r0:r0 + 64, :])

            for ch in range(NI // IC):
                i0 = ch * IC
                s0 = i0 % S
                xt = xt_pool.tile([128, ET, IC], bf16, name="xt", tag="xt")
                for ts in range(0, ET, 4):
                    nc.sync.dma_start(
                        xt[:, ts:ts + 4, :],
                        xT[ts * 128:(ts + 4) * 128, i0:i0 + IC].rearrange(
                            "(t p) i -> p t i", p=128))
                    if ch == 0 and ts < 12:
                        load_wq_quarter(ts + 4)
                for m in range(2):
                    q_ps = proj_ps.tile([128, IC], f32, name="pps",
                                        tag="proj")
                    for t in range(ET):
                        nc.tensor.matmul(
                            q_ps[:, :],
                            wq_sb[:, t, m * 128:(m + 1) * 128],
                            xt[:, t, :],
                            start=(t == 0), stop=(t == ET - 1))
                    rope(q_ps, 128, s0,
                         [(qt_sb[0:64, 2 * m, i0:i0 + IC], 0),
                          (qt_sb[0:64, 2 * m + 1, i0:i0 + IC], 64)])
                kv_ps = proj_ps.tile([128, IC], f32, name="pps", tag="proj")
                for t in range(ET):
                    nc.tensor.matmul(
                        kv_ps[:, :],
                        wkv_sb[:, t, :],
                        xt[:, t, :],
                        start=(t == 0), stop=(t == ET - 1))
                rope(kv_ps, 64, s0, [(kt_sb[0:64, i0:i0 + IC], 0)])
                vs = vs_pool.tile([64, IC], bf16, name="vs", tag="vs")
                nc.scalar.copy(vs[:, :], kv_ps[64:128, :])
                for j in range(IC // 128):
                    kidx = (i0 + j * 128) // 128
                    vt_ps = vt_ps_pool.tile([128, HD], bf16, name="vt",
                                            tag="vt")
                    nc.tensor.transpose(vt_ps[:, :],
                                        vs[:, j * 128:(j + 1) * 128],
                                        ident[0:64, 0:64])
                    nc.vector.tensor_copy(vaug[kidx][:, HD:2 * HD], vt_ps[:, :])

        wo_pool = persist.enter_context(tc.tile_pool(name="wo", bufs=1))
        wo_sb = wo_pool.tile([128, ET, E], bf16, name="wo_sb", tag="wo")
        nc.scalar.dma_start(wo_sb[:, :, :],
                            woT.rearrange("(t p) o -> p t o", p=128))
        ct_pool = persist.enter_context(tc.tile_pool(name="ct", bufs=1))
        ct = [ct_pool.tile([128, NCORES, TOK], bf16, name=f"ct{m}",
                           tag=f"ct{m}") for m in range(2)]

        # ---- phase 3: attention (head-pair outer so the pair-0 AllToAll
        #      overlaps pair-1 compute)
        with ExitStack() as ph3:
            scores_ps = ph3.enter_context(
                tc.tile_pool(name="scores_ps", bufs=2, space="PSUM"))
            ctx_ps_pool = ph3.enter_context(
                tc.tile_pool(name="ctx_ps", bufs=2, space="PSUM"))
            et_pool = ph3.enter_context(tc.tile_pool(name="et", bufs=4))
            rc_pool = ph3.enter_context(tc.tile_pool(name="rc", bufs=3))
            rb_pool = ph3.enter_context(tc.tile_pool(name="rb", bufs=3))
            st_pool = ph3.enter_context(tc.tile_pool(name="st", bufs=4))

            # persistent band tiles with pre-zeroed garbage region so band
            # exps write only [q0:] and ctx can read the full tile
            e_band = [et_pool.tile([128, 2, QB], bf16, name=f"eb{j}",
                                   tag=f"eb{j}", bufs=1)
                      for j in range(QB // KB)]
            for j in range(1, QB // KB):
                nc.gpsimd.memset(e_band[j][:, :, 0:j * KB], 0.0)

            jobs = []
            for m in range(2):
                for b in range(B):
                    for qt in range(NQT):
                        nkt = (qt + 1) * (QB // KB)
                        for kt in range(nkt):
                            jobs.append((m, b, qt, kt, nkt))

            def emit_scores(job):
                m, b, qt, kt, nkt = job
                sl = b * S + qt * QB
                j = kt - qt * (QB // KB)
                kp = b * S + kt * KB
                s_ps = scores_ps.tile([128, 2, QB], f32, name="sps",
                                      tag="sps")
                if j < 0:
                    e_t = et_pool.tile([128, 2, QB], bf16, name="et",
                                       tag="et")
                    for h in range(2):
                        nc.tensor.matmul(
                            s_ps[:, h, :],
                            kt_sb[0:64, kp:kp + KB],
                            qt_sb[0:64, 2 * m + h, sl:sl + QB],
                            start=True, stop=True)
                    nc.scalar.activation(e_t[:, :, :], s_ps[:, :, :], Exp,
                                         scale=scale)
                else:
                    e_t = e_band[j]
                    q0 = j * KB
                    for h in range(2):
                        nc.tensor.matmul(
                            s_ps[:, h, q0:QB],
                            kt_sb[0:64, kp:kp + KB],
                            qt_sb[0:64, 2 * m + h, sl + q0:sl + QB],
                            start=True, stop=True)
                    nc.scalar.activation(e_t[:, :, q0:QB],
                                         s_ps[:, :, q0:QB], Exp,
                                         scale=scale)
                    nc.vector.tensor_mul(e_t[:, :, q0:q0 + KB],
                                         e_t[:, :, q0:q0 + KB],
                                         tri[:, :, :])
                return e_t

            ctx_cur = [None]

            def emit_ctx(job, e_t):
                m, b, qt, kt, nkt = job
                if kt == 0:
                    ctx_cur[0] = ctx_ps_pool.tile([128, 2, QB], f32,
                                                  name="ctx", tag="ctx")
                ctx_ps = ctx_cur[0]
                j = kt - qt * (QB // KB)
                q0 = max(j, 0) * KB  # zero region of band tiles: skip it
                for h in range(2):
                    nc.tensor.matmul(
                        ctx_ps[:, h, q0:QB],
                        vaug[b * SKT + kt][:, :],
                        e_t[:, h, q0:QB],
                        start=(kt == 0), stop=(kt == nkt - 1),
                        skip_group_check=(q0 > 0))
                if kt != nkt - 1:
                    return
                # normalize by the ones-column row + stage for A2A
                rc = rc_pool.tile([1, 2, QB], f32, name="rc", tag="rc")
                nc.vector.reciprocal_approx_fast(rc[:, :, :],
                                                 ctx_ps[0:1, :, :])
                rb = rb_pool.tile([64, 2, QB], f32, name="rb", tag="rb")
                nc.gpsimd.partition_broadcast(rb[:, :, :], rc[:, :, :])
                stage = st_pool.tile([128, QB], bf16, name="st", tag="st")
                nc.vector.tensor_mul(stage[0:64, :],
                                     ctx_ps[HD:2 * HD, 0, :], rb[:, 0, :])
                nc.vector.tensor_mul(stage[64:128, :],
                                     ctx_ps[HD:2 * HD, 1, :], rb[:, 1, :])
                nc.sync.dma_start(cc_in[m][b * NQT + qt, :, :], stage[:, :])
                if (b, qt) == (B - 1, NQT - 1):
                    # ---- phase 4: AllToAll for this head-pair, then start
                    # loading the re-sharded ctx while attention continues
                    nc.gpsimd.collective_compute(
                        "AllToAll", mybir.AluOpType.bypass,
                        replica_groups=[list(range(NCORES))],
                        ins=[cc_in[m][:, :, :]],
                        outs=[cc_out[m][:, :, :]])
                    for dh in range(2):
                        d0 = dh * (NCORES // 2)
                        nc.sync.dma_start(
                            ct[m][:, d0:d0 + NCORES // 2, :],
                            cc_out[m][d0:d0 + NCORES // 2].rearrange(
                                "s p n -> p s n"))

            # software-pipelined emission: scores(i+1) lands on the PE queue
            # before ctx(i), so the PE never blocks waiting on exp(i)
            prev = None
            for job in jobs:
                e_t = emit_scores(job)
                if prev is not None:
                    emit_ctx(*prev)
                prev = (job, e_t)
            emit_ctx(*prev)

        # ---- phase 5: output projection for this core's 512-token slice
        with ExitStack() as ph5:
            out_ps_pool = ph5.enter_context(
                tc.tile_pool(name="out_ps", bufs=2, space="PSUM"))
            ob_pool = ph5.enter_context(tc.tile_pool(name="ob", bufs=2))

            pp_pool = ph5.enter_context(tc.tile_pool(name="pp", bufs=1))
            partial = [pp_pool.tile([128, E], bf16, name=f"pp{c}",
                                    tag=f"pp{c}") for c in range(4)]
            # pass 1: pair-0 contributions for all four token chunks; runs
            # while the pair-1 AllToAll is still in flight
            for c in range(4):
                o_ps = out_ps_pool.tile([128, E], f32, name="o1", tag="ops")
                for d in range(NCORES):
                    for o in range(E // 512):
                        nc.tensor.matmul(
                            o_ps[:, o * 512:(o + 1) * 512],
                            ct[0][:, d, c * 128:(c + 1) * 128],
                            wo_sb[:, 2 * d, o * 512:(o + 1) * 512],
                            start=(d == 0), stop=(d == NCORES - 1))
                nc.scalar.copy(partial[c][:, 0:E // 2], o_ps[:, 0:E // 2])
                nc.vector.tensor_copy(partial[c][:, E // 2:E],
                                      o_ps[:, E // 2:E])
            # pass 2: pair-1 contributions + staged partial -> output
            for c in range(4):
                o_ps = out_ps_pool.tile([128, E], f32, name="o2", tag="ops")
                for d in range(NCORES):
                    for o in range(E // 512):
                        nc.tensor.matmul(
                            o_ps[:, o * 512:(o + 1) * 512],
                            ct[1][:, d, c * 128:(c + 1) * 128],
                            wo_sb[:, 2 * d + 1, o * 512:(o + 1) * 512],
                            start=(d == 0), stop=(d == NCORES - 1))
                ob = ob_pool.tile([128, E], bf16, name="ob", tag="ob")
                for q in range(4):
                    ql, qh = q * (E // 4), (q + 1) * (E // 4)
                    nc.vector.tensor_add(ob[:, ql:qh], o_ps[:, ql:qh],
                                         partial[c][:, ql:qh])
                    nc.sync.dma_start(outT[c * 128:(c + 1) * 128, ql:qh],
                                      ob[:, ql:qh])

    nc.compile()
    return nc


def make_in_maps(cfg, x, cos, sin, Wq, Wk, Wv, Wo):
    """Host-side prep: transpose/slice full inputs into per-core maps."""
    import ml_dtypes
    B, S, E = cfg["B"], cfg["S"], cfg["E"]
    NH, NKV, HD, NCORES = cfg["NH"], cfg["NKV"], cfg["HD"], cfg["ncores"]
    HPC = NH // NCORES
    QH = HPC * HD
    KVPC = NKV // NCORES
    bf = ml_dtypes.bfloat16

    x = np.asarray(x, dtype=np.float32)
    cos = np.asarray(cos, dtype=np.float32)
    sin = np.asarray(sin, dtype=np.float32)
    Wq = np.asarray(Wq, dtype=np.float32)
    Wk = np.asarray(Wk, dtype=np.float32)
    Wv = np.asarray(Wv, dtype=np.float32)
    Wo = np.asarray(Wo, dtype=np.float32)

    xT = np.ascontiguousarray(x.reshape(B * S, E).T.astype(bf))
    cos_t = cos.T[:HD]                        # [64, S]
    cosT = np.ascontiguousarray(
        np.concatenate([cos_t, cos_t], axis=0).astype(bf))
    sin_t = sin.T[:HD].copy()
    sin_t[:HD // 2] *= -1.0                   # signed sin for rotate-half
    sinT = np.ascontiguousarray(
        np.concatenate([sin_t, sin_t], axis=0).astype(bf))
    woT = np.ascontiguousarray(Wo.T.astype(bf))  # full [E_in, E_out]

    in_maps = []
    for c in range(NCORES):
        qsl = slice(c * QH, (c + 1) * QH)
        ksl = slice(c * KVPC * HD, (c + 1) * KVPC * HD)
        wq = np.ascontiguousarray(Wq[qsl, :].T.astype(bf))
        wkv = np.ascontiguousarray(
            np.concatenate([Wk[ksl, :].T, Wv[ksl, :].T], axis=1).astype(bf))
        in_maps.append(dict(xT=xT, wqT=wq, wkvT=wkv, woT=woT,
                            cosT=cosT, sinT=sinT))
    return in_maps


def assemble_output(cfg, results):
    B, S, E = cfg["B"], cfg["S"], cfg["E"]
    out = np.concatenate([np.asarray(r["outT"]) for r in results], axis=0)
    return np.ascontiguousarray(out.astype(np.float32).reshape(B, S, E))


def kernel(x, mask, cos, sin, Wq, Wk, Wv, Wo):
    global LAST_RESULTS, _CACHED_NC
    _ensure_concourse()
    from concourse import bass_utils

    cfg = FULL_CFG
    if _CACHED_NC is None:
        _CACHED_NC = build_gqa(cfg)
    nc = _CACHED_NC
    in_maps = make_in_maps(cfg, x, cos, sin, Wq, Wk, Wv, Wo)
    res = bass_utils.run_bass_kernel_spmd(
        nc, in_maps, core_ids=list(range(cfg["ncores"])))
    LAST_RESULTS = res
    return assemble_output(cfg, res.results)


# revision 21
# speedup vs baseline: 1.0171x; 1.0171x over previous
"""GroupedQueryAttention TRN2 Bass kernel (v2).

Strategy (8 NeuronCores, tensor-parallel over heads):
  - Each core owns 4 q-heads (one kv head, GQA group of 4), all tokens.
  - Phase 1: QKV projection (bf16 matmuls, N=512 chunks) + fused RoPE.
    Q stored [64, 4 heads, NI] so scores batch 2 heads per matmul.
  - Phase 3: causal flash-style attention per (head-pair, batch, q-stripe):
      S = K_blk^T.T @ Q(2 heads)  -> exp on ACT (causally trimmed)
      ctx^T += V_aug.T @ exp  (V augmented with ones column so the softmax
      denominator falls out of the same matmul); normalize via reciprocal +
      partition broadcast fused into the bf16 staging store.
  - Phase 4: two 1 MB AllToAlls (one per head-pair) re-shard from
    head-sharded to token-sharded; the first overlaps pair-1 attention.
  - Phase 5: out = ctx_tok^T stationary x full-Wo moving (N=2048 matmuls),
    each core emits out[token-slice 512, 2048].
  - Host concatenates the 8 token slices.
"""

import os
import sys

import numpy as np


def _ensure_concourse():
    try:
        import concourse.bass  # noqa: F401
    except ImportError:
        for p in ("/opt/trn_rl_repo", "/root/.axon_site/_ro/trn_rl_repo"):
            if os.path.isdir(p) and p not in sys.path:
                sys.path.insert(0, p)
        import concourse.bass  # noqa: F401


FULL_CFG = dict(B=2, S=2048, E=2048, NH=32, NKV=8, HD=64, ncores=8, IC=512)

LAST_RESULTS = None
_CACHED_NC = None


def build_gqa(cfg):
    """Build the Bass module for one core's SPMD program. Returns nc."""
    _ensure_concourse()
    from contextlib import ExitStack

    import concourse.mybir as mybir
    import concourse.tile as tile
    from concourse import bacc
    from concourse.masks import make_identity

    dt = mybir.dt
    f32 = dt.float32
    bf16 = dt.bfloat16
    Exp = mybir.ActivationFunctionType.Exp

    B, S, E = cfg["B"], cfg["S"], cfg["E"]
    NH, NKV, HD = cfg["NH"], cfg["NKV"], cfg["HD"]
    NCORES = cfg["ncores"]
    HPC = NH // NCORES          # 4 q heads per core
    assert HPC == 4 and HD == 64
    QH = HPC * HD               # 256 ctx rows per core
    KVD = 2 * HD                # 128 packed K|V projection width
    NI = B * S                  # 4096 tokens
    ET = E // 128               # 16 contraction tiles
    IC = cfg["IC"]              # phase-1 token chunk (1024)
    QB = 512                    # attention q stripe
    KB = 128                    # attention k block
    NQT = S // QB               # 4 stripes per batch
    SKT = S // KB               # 16 k tiles per batch
    NKTILES = NI // KB          # 32 k tiles
    TOK = NI // NCORES          # 512-token output slice per core
    scale = 1.0 / float(np.sqrt(HD))

    nc = bacc.Bacc("TRN2", target_bir_lowering=False, debug=False,
                   num_devices=NCORES)

    xT = nc.dram_tensor("xT", [E, NI], bf16, kind="ExternalInput").ap()
    wqT = nc.dram_tensor("wqT", [E, QH], bf16, kind="ExternalInput").ap()
    wkvT = nc.dram_tensor("wkvT", [E, KVD], bf16, kind="ExternalInput").ap()
    woT = nc.dram_tensor("woT", [E, E], bf16, kind="ExternalInput").ap()
    cosT = nc.dram_tensor("cosT", [128, S], bf16, kind="ExternalInput").ap()
    sinT = nc.dram_tensor("sinT", [128, S], bf16, kind="ExternalInput").ap()
    outT = nc.dram_tensor("outT", [TOK, E], bf16, kind="ExternalOutput").ap()

    with tile.TileContext(nc) as tc, ExitStack() as persist:
        const = persist.enter_context(tc.tile_pool(name="const", bufs=1))
        qt_pool = persist.enter_context(tc.tile_pool(name="qt", bufs=1))
        kt_pool = persist.enter_context(tc.tile_pool(name="kt", bufs=1))
        vaug_pool = persist.enter_context(tc.tile_pool(name="vaug", bufs=1))
        dram = persist.enter_context(
            tc.tile_pool(name="dram", bufs=1, space="DRAM"))

        ident = const.tile([128, 128], bf16, name="ident", tag="ident")
        make_identity(nc, ident[:, :])
        # wq + x chunks go on the sync DGE ring; cos/sin/wo on the scalar
        # ring so the 8 MB wo load does not delay the first matmul.
        wq_sb = const.tile([128, ET, QH], bf16, name="wq_sb", tag="wq")
        for ts in range(0, ET, 4):
            nc.sync.dma_start(
                wq_sb[:, ts:ts + 4, :],
                wqT[ts * 128:(ts + 4) * 128, :].rearrange(
                    "(t p) o -> p t o", p=128))
        wkv_sb = const.tile([128, ET, KVD], bf16, name="wkv_sb", tag="wkv")
        nc.sync.dma_start(wkv_sb[:, :, :],
                          wkvT.rearrange("(t p) o -> p t o", p=128))
        cos_sb = const.tile([128, S], bf16, name="cos_sb", tag="cos")
        nc.scalar.dma_start(cos_sb[:, :], cosT)
        sin_sb = const.tile([128, S], bf16, name="sin_sb", tag="sin")
        nc.scalar.dma_start(sin_sb[:, :], sinT)
        # triangular causal mask for the diagonal 128-block, dup for 2 heads
        tri = const.tile([128, 2, 128], bf16, name="tri", tag="tri")
        nc.gpsimd.memset(tri[:, :, :], 1.0)
        nc.gpsimd.affine_select(
            out=tri[:, :, :], in_=tri[:, :, :],
            pattern=[[0, 2], [1, 128]], compare_op=mybir.AluOpType.is_ge,
            fill=0.0, base=0, channel_multiplier=-1)

        # persistent activations
        qt_sb = qt_pool.tile([64, HPC, NI], bf16, name="qt", tag="qt")
        kt_sb = kt_pool.tile([64, NI], bf16, name="kt", tag="kt")
        vaug = [vaug_pool.tile([128, 2 * HD], bf16, name=f"va{k}",
                               tag=f"va{k}")
                for k in range(NKTILES)]
        for k in range(NKTILES):
            nc.vector.memset(vaug[k][:, :], 0.0)
            nc.vector.memset(vaug[k][:, 0:1], 1.0)

        # collective buffers: per head-pair m, [slice, 128 rows, 512 tokens]
        cc_in = [dram.tile([NCORES, 128, TOK], bf16, name=f"cc_in{m}",
                           tag=f"ccin{m}") for m in range(2)]
        cc_out = [dram.tile([NCORES, 128, TOK], bf16, name=f"cc_out{m}",
                            tag=f"ccout{m}") for m in range(2)]

        # ---- phase 1: QKV projection + RoPE + V transpose
        with ExitStack() as ph1:
            xt_pool = ph1.enter_context(tc.tile_pool(name="xt", bufs=2))
            proj_ps = ph1.enter_context(
                tc.tile_pool(name="proj_ps", bufs=3, space="PSUM"))
            vt_ps_pool = ph1.enter_context(
                tc.tile_pool(name="vt_ps", bufs=2, space="PSUM"))
            rope_pool = ph1.enter_context(tc.tile_pool(name="rope", bufs=4))
            vs_pool = ph1.enter_context(tc.tile_pool(name="vs", bufs=3))

            def rope(src_ps, parts, s0, dsts):
                # dsts: list of (out_ap, row0) pairs covering src rows
                t1 = rope_pool.tile([128, IC], bf16, name="t1", tag="t1")
                sw = rope_pool.tile([128, IC], bf16, name="sw", tag="sw")
                for h0 in range(0, parts, 64):
                    nc.scalar.copy(sw[h0:h0 + 32, :],
                                   src_ps[h0 + 32:h0 + 64, :])
                    nc.scalar.copy(sw[h0 + 32:h0 + 64, :],
                                   src_ps[h0:h0 + 32, :])
                nc.vector.tensor_mul(t1[:parts, :], src_ps[:parts, :],
                                     cos_sb[:parts, s0:s0 + IC])
                nc.vector.tensor_mul(sw[:parts, :], sw[:parts, :],
                                     sin_sb[:parts, s0:s0 + IC])
                for out_ap, r0 in dsts:
                    nc.vector.tensor_add(out_ap, t1[r0:r0 + 64, :],
                                         sw[r0:r0 + 64, :])

            for ch in range(NI // IC):
                i0 = ch * IC
                s0 = i0 % S
                xt = xt_pool.tile([128, ET, IC], bf16, name="xt", tag="xt")
                for ts in range(0, ET, 4):
                    nc.sync.dma_start(
                        xt[:, ts:ts + 4, :],
                        xT[ts * 128:(ts + 4) * 128, i0:i0 + IC].rearrange(
                            "(t p) i -> p t i", p=128))
                for m in range(2):
                    q_ps = proj_ps.tile([128, IC], f32, name="pps",
                                        tag="proj")
                    for t in range(ET):
                        nc.tensor.matmul(
                            q_ps[:, :],
                            wq_sb[:, t, m * 128:(m + 1) * 128],
                            xt[:, t, :],
                            start=(t == 0), stop=(t == ET - 1))
                    rope(q_ps, 128, s0,
                         [(qt_sb[0:64, 2 * m, i0:i0 + IC], 0),
                          (qt_sb[0:64, 2 * m + 1, i0:i0 + IC], 64)])
                kv_ps = proj_ps.tile([128, IC], f32, name="pps", tag="proj")
                for t in range(ET):
                    nc.tensor.matmul(
                        kv_ps[:, :],
                        wkv_sb[:, t, :],
                        xt[:, t, :],
                        start=(t == 0), stop=(t == ET - 1))
                rope(kv_ps, 64, s0, [(kt_sb[0:64, i0:i0 + IC], 0)])
                vs = vs_pool.tile([64, IC], bf16, name="vs", tag="vs")
                nc.scalar.copy(vs[:, :], kv_ps[64:128, :])
                for j in range(IC // 128):
                    kidx = (i0 + j * 128) // 128
                    vt_ps = vt_ps_pool.tile([128, HD], bf16, name="vt",
                                            tag="vt")
                    nc.tensor.transpose(vt_ps[:, :],
                                        vs[:, j * 128:(j + 1) * 128],
                                        ident[0:64, 0:64])
                    nc.vector.tensor_copy(vaug[kidx][:, HD:2 * HD], vt_ps[:, :])

        wo_pool = persist.enter_context(tc.tile_pool(name="wo", bufs=1))
        wo_sb = wo_pool.tile([128, ET, E], bf16, name="wo_sb", tag="wo")
        nc.scalar.dma_start(wo_sb[:, :, :],
                            woT.rearrange("(t p) o -> p t o", p=128))
        ct_pool = persist.enter_context(tc.tile_pool(name="ct", bufs=1))
        ct = [ct_pool.tile([128, NCORES, TOK], bf16, name=f"ct{m}",
                           tag=f"ct{m}") for m in range(2)]

        # ---- phase 3: attention (head-pair outer so the pair-0 AllToAll
        #      overlaps pair-1 compute)
        with ExitStack() as ph3:
            scores_ps = ph3.enter_context(
                tc.tile_pool(name="scores_ps", bufs=2, space="PSUM"))
            ctx_ps_pool = ph3.enter_context(
                tc.tile_pool(name="ctx_ps", bufs=2, space="PSUM"))
            et_pool = ph3.enter_context(tc.tile_pool(name="et", bufs=4))
            rc_pool = ph3.enter_context(tc.tile_pool(name="rc", bufs=3))
            rb_pool = ph3.enter_context(tc.tile_pool(name="rb", bufs=3))
            st_pool = ph3.enter_context(tc.tile_pool(name="st", bufs=4))

            # persistent band tiles with pre-zeroed garbage region so band
            # exps write only [q0:] and ctx can read the full tile
            e_band = [et_pool.tile([128, 2, QB], bf16, name=f"eb{j}",
                                   tag=f"eb{j}", bufs=1)
                      for j in range(QB // KB)]
            for j in range(1, QB // KB):
                nc.gpsimd.memset(e_band[j][:, :, 0:j * KB], 0.0)

            jobs = []
            for m in range(2):
                for b in range(B):
                    for qt in range(NQT):
                        nkt = (qt + 1) * (QB // KB)
                        for kt in range(nkt):
                            jobs.append((m, b, qt, kt, nkt))

            def emit_scores(job):
                m, b, qt, kt, nkt = job
                sl = b * S + qt * QB
                j = kt - qt * (QB // KB)
                kp = b * S + kt * KB
                s_ps = scores_ps.tile([128, 2, QB], f32, name="sps",
                                      tag="sps")
                if j < 0:
                    e_t = et_pool.tile([128, 2, QB], bf16, name="et",
                                       tag="et")
                    for h in range(2):
                        nc.tensor.matmul(
                            s_ps[:, h, :],
                            kt_sb[0:64, kp:kp + KB],
                            qt_sb[0:64, 2 * m + h, sl:sl + QB],
                            start=True, stop=True)
                    nc.scalar.activation(e_t[:, :, :], s_ps[:, :, :], Exp,
                                         scale=scale)
                else:
                    e_t = e_band[j]
                    q0 = j * KB
                    for h in range(2):
                        nc.tensor.matmul(
                            s_ps[:, h, q0:QB],
                            kt_sb[0:64, kp:kp + KB],
                            qt_sb[0:64, 2 * m + h, sl + q0:sl + QB],
                            start=True, stop=True)
                    nc.scalar.activation(e_t[:, :, q0:QB],
                                         s_ps[:, :, q0:QB], Exp,
                                         scale=scale)
                    nc.vector.tensor_mul(e_t[:, :, q0:q0 + KB],
                                         e_t[:, :, q0:q0 + KB],
                                         tri[:, :, :])
                return e_t

            ctx_cur = [None]

            def emit_ctx(job, e_t):
                m, b, qt, kt, nkt = job
                if kt == 0:
                    ctx_cur[0] = ctx_ps_pool.tile([128, 2, QB], f32,
                                                  name="ctx", tag="ctx")
                ctx_ps = ctx_cur[0]
                j = kt - qt * (QB // KB)
                q0 = max(j, 0) * KB  # zero region of band tiles: skip it
                for h in range(2):
                    nc.tensor.matmul(
                        ctx_ps[:, h, q0:QB],
                        vaug[b * SKT + kt][:, :],
                        e_t[:, h, q0:QB],
                        start=(kt == 0), stop=(kt == nkt - 1),
                        skip_group_check=(q0 > 0))
                if kt != nkt - 1:
                    return
                # normalize by the ones-column row + stage for A2A
                rc = rc_pool.tile([1, 2, QB], f32, name="rc", tag="rc")
                nc.vector.reciprocal_approx_fast(rc[:, :, :],
                                                 ctx_ps[0:1, :, :])
                rb = rb_pool.tile([64, 2, QB], f32, name="rb", tag="rb")
                nc.gpsimd.partition_broadcast(rb[:, :, :], rc[:, :, :])
                stage = st_pool.tile([128, QB], bf16, name="st", tag="st")
                nc.vector.tensor_mul(stage[0:64, :],
                                     ctx_ps[HD:2 * HD, 0, :], rb[:, 0, :])
                nc.vector.tensor_mul(stage[64:128, :],
                                     ctx_ps[HD:2 * HD, 1, :], rb[:, 1, :])
                nc.sync.dma_start(cc_in[m][b * NQT + qt, :, :], stage[:, :])
                if (b, qt) == (B - 1, NQT - 1):
                    # ---- phase 4: AllToAll for this head-pair, then start
                    # loading the re-sharded ctx while attention continues
                    nc.gpsimd.collective_compute(
                        "AllToAll", mybir.AluOpType.bypass,
                        replica_groups=[list(range(NCORES))],
                        ins=[cc_in[m][:, :, :]],
                        outs=[cc_out[m][:, :, :]])
                    for dh in range(2):
                        d0 = dh * (NCORES // 2)
                        nc.sync.dma_start(
                            ct[m][:, d0:d0 + NCORES // 2, :],
                            cc_out[m][d0:d0 + NCORES // 2].rearrange(
                                "s p n -> p s n"))

            # software-pipelined emission: scores(i+1) lands on the PE queue
            # before ctx(i), so the PE never blocks waiting on exp(i)
            prev = None
            for job in jobs:
                e_t = emit_scores(job)
                if prev is not None:
                    emit_ctx(*prev)
                prev = (job, e_t)
            emit_ctx(*prev)

        # ---- phase 5: output projection for this core's 512-token slice
        with ExitStack() as ph5:
            out_ps_pool = ph5.enter_context(
                tc.tile_pool(name="out_ps", bufs=2, space="PSUM"))
            ob_pool = ph5.enter_context(tc.tile_pool(name="ob", bufs=2))

            pp_pool = ph5.enter_context(tc.tile_pool(name="pp", bufs=1))
            partial = [pp_pool.tile([128, E], bf16, name=f"pp{c}",
                                    tag=f"pp{c}") for c in range(4)]
            # pass 1: pair-0 contributions for all four token chunks; runs
            # while the pair-1 AllToAll is still in flight
            for c in range(4):
                o_ps = out_ps_pool.tile([128, E], f32, name="o1", tag="ops")
                for d in range(NCORES):
                    for o in range(E // 512):
                        nc.tensor.matmul(
                            o_ps[:, o * 512:(o + 1) * 512],
                            ct[0][:, d, c * 128:(c + 1) * 128],
                            wo_sb[:, 2 * d, o * 512:(o + 1) * 512],
                            start=(d == 0), stop=(d == NCORES - 1))
                nc.scalar.copy(partial[c][:, 0:E // 2], o_ps[:, 0:E // 2])
                nc.vector.tensor_copy(partial[c][:, E // 2:E],
                                      o_ps[:, E // 2:E])
            # pass 2: pair-1 contributions + staged partial -> output
            for c in range(4):
                o_ps = out_ps_pool.tile([128, E], f32, name="o2", tag="ops")
                for d in range(NCORES):
                    for o in range(E // 512):
                        nc.tensor.matmul(
                            o_ps[:, o * 512:(o + 1) * 512],
                            ct[1][:, d, c * 128:(c + 1) * 128],
                            wo_sb[:, 2 * d + 1, o * 512:(o + 1) * 512],
                            start=(d == 0), stop=(d == NCORES - 1))
                ob = ob_pool.tile([128, E], bf16, name="ob", tag="ob")
                nc.vector.tensor_add(ob[:, 0:E // 2], o_ps[:, 0:E // 2],
                                     partial[c][:, 0:E // 2])
                nc.sync.dma_start(outT[c * 128:(c + 1) * 128, 0:E // 2],
                                  ob[:, 0:E // 2])
                nc.vector.tensor_add(ob[:, E // 2:E], o_ps[:, E // 2:E],
                                     partial[c][:, E // 2:E])
                nc.sync.dma_start(outT[c * 128:(c + 1) * 128, E // 2:E],
                                  ob[:, E // 2:E])

    nc.compile()
    return nc


def make_in_maps(cfg, x, cos, sin, Wq, Wk, Wv, Wo):
    """Host-side prep: transpose/slice full inputs into per-core maps."""
    import ml_dtypes
    B, S, E = cfg["B"], cfg["S"], cfg["E"]
    NH, NKV, HD, NCORES = cfg["NH"], cfg["NKV"], cfg["HD"], cfg["ncores"]
    HPC = NH // NCORES
    QH = HPC * HD
    KVPC = NKV // NCORES
    bf = ml_dtypes.bfloat16

    x = np.asarray(x, dtype=np.float32)
    cos = np.asarray(cos, dtype=np.float32)
    sin = np.asarray(sin, dtype=np.float32)
    Wq = np.asarray(Wq, dtype=np.float32)
    Wk = np.asarray(Wk, dtype=np.float32)
    Wv = np.asarray(Wv, dtype=np.float32)
    Wo = np.asarray(Wo, dtype=np.float32)

    xT = np.ascontiguousarray(x.reshape(B * S, E).T.astype(bf))
    cos_t = cos.T[:HD]                        # [64, S]
    cosT = np.ascontiguousarray(
        np.concatenate([cos_t, cos_t], axis=0).astype(bf))
    sin_t = sin.T[:HD].copy()
    sin_t[:HD // 2] *= -1.0                   # signed sin for rotate-half
    sinT = np.ascontiguousarray(
        np.concatenate([sin_t, sin_t], axis=0).astype(bf))
    woT = np.ascontiguousarray(Wo.T.astype(bf))  # full [E_in, E_out]

    in_maps = []
    for c in range(NCORES):
        qsl = slice(c * QH, (c + 1) * QH)
        ksl = slice(c * KVPC * HD, (c + 1) * KVPC * HD)
        wq = np.ascontiguousarray(Wq[qsl, :].T.astype(bf))
        wkv = np.ascontiguousarray(
            np.concatenate([Wk[ksl, :].T, Wv[ksl, :].T], axis=1).astype(bf))
        in_maps.append(dict(xT=xT, wqT=wq, wkvT=wkv, woT=woT,
                            cosT=cosT, sinT=sinT))
    return in_maps


def assemble_output(cfg, results):
    B, S, E = cfg["B"], cfg["S"], cfg["E"]
    out = np.concatenate([np.asarray(r["outT"]) for r in results], axis=0)
    return np.ascontiguousarray(out.astype(np.float32).reshape(B, S, E))


def kernel(x, mask, cos, sin, Wq, Wk, Wv, Wo):
    global LAST_RESULTS, _CACHED_NC
    _ensure_concourse()
    from concourse import bass_utils

    cfg = FULL_CFG
    if _CACHED_NC is None:
        _CACHED_NC = build_gqa(cfg)
    nc = _CACHED_NC
    in_maps = make_in_maps(cfg, x, cos, sin, Wq, Wk, Wv, Wo)
    res = bass_utils.run_bass_kernel_spmd(
        nc, in_maps, core_ids=list(range(cfg["ncores"])))
    LAST_RESULTS = res
    return assemble_output(cfg, res.results)
